# revision 1
# baseline (speedup 1.0000x reference)
"""Trainium2 Bass kernel for nn_MultiHeadAttnCoupling.

Reference computation (B=4, N=128, D=32768, heads=8, seq=64, d_tensor=64):
    Q = (z @ Wq + bq).reshape(B,N,H,S,DT)   # per (b,n): attention over S
    K = (x @ Wk + bk).reshape(...)
    V = (x @ Wv + bv).reshape(...)
    out = softmax(Q K^T / 8) V  -> reshape -> @ Wo + bo

Sharding: head-parallel over 8 cores (one head per core). Each core gets its
head's weight slices and computes a partial output (contribution of its head's
d-slice through Wo); the host sums the 8 partials and adds bo.

Per-core scheme (layouts validated in numpy, primitives probed on HW):
  - projections computed transposed: chunks [128=(2 s-values x dt), tokens],
    evicted into "slab" SBUF tensors [64(part)=dt, s, t] (Q,K) and
    [64(part)=s_k, dt(+ones), t] (V, dt-major via host column permutation);
    bias folded into the eviction ops (ACT Identity / DVE tensor_scalar).
  - attention in groups of 7 tokens (one PSUM bank wide): per-token matmuls
    with strided APs straight off the slabs (scoresT = k @ q^T; attn@V with a
    built-in ones slab producing the softmax denominator as column 64), one
    batched exp on ScalarE per group, normalization via two broadcast-divide
    tensor_tensor ops that also scatter into the dt-major OT' layout.
  - output projection uses dt-major-permuted Wo (host side), producing
    partialT [c, t] per core.

Known pitfall (bisected on HW): an accumulating K-split matmul pair
(tile_position row packing) whose lhsT was written by a compute engine
deadlocks the device. All attention matmuls therefore use plain K=64.
"""

import numpy as np
import ml_dtypes

B, N = 4, 128
INPUT_SIZE, Z_SIZE = 512, 256
DT, H, S = 64, 8, 64
D = DT * H * S            # 32768
DH = S * DT               # 4096 per head
T = B * N                 # 512 tokens
TB = 256                  # token block
NBLK = T // TB
CH = DH // 128            # 32 chunks per projection
KCQ = Z_SIZE // 128       # 2
KCX = INPUT_SIZE // 128   # 4
CT = INPUT_SIZE // 128    # 4 output col tiles
G = 7                     # tokens per attention group (PSUM bank width)
CB = 4                    # chunks per weight DMA batch

_bf16 = ml_dtypes.bfloat16

_cache = {}


def _build_nc(reps=1):
    import concourse.mybir as mybir
    import concourse.tile as tile
    from concourse import bacc

    f32, bf16 = mybir.dt.float32, mybir.dt.bfloat16
    AF = mybir.ActivationFunctionType
    MUL = mybir.AluOpType.mult

    nc = bacc.Bacc("TRN2", target_bir_lowering=False, debug=False)

    zt_d = nc.dram_tensor("zt", [128, KCQ, T], bf16, kind="ExternalInput")
    xt_d = nc.dram_tensor("xt", [128, KCX, T], bf16, kind="ExternalInput")
    wq_d = nc.dram_tensor("wq", [128, CH, KCQ * 128], bf16, kind="ExternalInput")
    wk_d = nc.dram_tensor("wk", [128, CH, KCX * 128], bf16, kind="ExternalInput")
    wv_d = nc.dram_tensor("wv", [128, CH, KCX * 128], bf16, kind="ExternalInput")
    wo_d = nc.dram_tensor("wo", [128, CH, CT * 128], bf16, kind="ExternalInput")
    bq_d = nc.dram_tensor("bq", [128, CH], f32, kind="ExternalInput")
    bk_d = nc.dram_tensor("bk", [128, CH], f32, kind="ExternalInput")
    bv_d = nc.dram_tensor("bv", [128, CH], f32, kind="ExternalInput")
    pt_d = nc.dram_tensor("pt", [INPUT_SIZE, T], f32, kind="ExternalOutput")

    with tile.TileContext(nc) as tc:
        with (
            tc.tile_pool(name="acts", bufs=1) as acts_pool,
            tc.tile_pool(name="slabs", bufs=1) as slab_pool,
            tc.tile_pool(name="wts", bufs=3) as wts_pool,
            tc.tile_pool(name="small", bufs=4) as small_pool,
            tc.tile_pool(name="osb", bufs=2) as osb_pool,
            tc.tile_pool(name="psb", bufs=4, space="PSUM") as psb_pool,
            tc.tile_pool(name="pss", bufs=2, space="PSUM") as pss_pool,
            tc.tile_pool(name="pso", bufs=2, space="PSUM") as pso_pool,
        ):
            # resident activations and biases
            zt = acts_pool.tile([128, KCQ, T], bf16, tag="zt")
            xt = acts_pool.tile([128, KCX, T], bf16, tag="xt")
            nc.sync.dma_start(zt[:], zt_d[:])
            nc.sync.dma_start(xt[:], xt_d[:])
            bq = acts_pool.tile([128, CH], f32, tag="bq")
            bk = acts_pool.tile([128, CH], f32, tag="bk")
            bv = acts_pool.tile([128, CH], f32, tag="bv")
            nc.sync.dma_start(bq[:], bq_d[:])
            nc.sync.dma_start(bk[:], bk_d[:])
            nc.sync.dma_start(bv[:], bv_d[:])

            for rep in range(reps):
              for blk in range(NBLK):
                tsl = slice(blk * TB, (blk + 1) * TB)
                qts = slab_pool.tile([64, S, TB], bf16, tag="qts")
                kts = slab_pool.tile([64, S, TB], bf16, tag="kts")
                vs = slab_pool.tile([64, DT + 1, TB], bf16, tag="vs")
                otp = slab_pool.tile([128, CH, TB], bf16, tag="otp")
                nc.vector.memset(vs[:, DT, :], 1.0)

                # ---- projections ----
                for (w_d, wtag, nkc, act, bias, slab, hi_on_act) in (
                    (wq_d, "wq", KCQ, zt, bq, qts, False),
                    (wk_d, "wk", KCX, xt, bk, kts, False),
                    (wv_d, "wv", KCX, xt, bv, vs, False),
                ):
                    for c4 in range(CH // CB):
                        wt = wts_pool.tile([128, CB, nkc, 128], bf16, tag=wtag)
                        nc.sync.dma_start(
                            wt[:], w_d[:, CB * c4:CB * (c4 + 1), :].rearrange(
                                "p c (kc m) -> p c kc m", m=128))
                        for ci in range(CB):
                            c = CB * c4 + ci
                            ps = psb_pool.tile([128, TB], f32, tag="big",
                                               name=f"pj{rep}{blk}{wtag}{c}")
                            for kc in range(nkc):
                                nc.tensor.matmul(
                                    ps[:], wt[:, ci, kc, :], act[:, kc, tsl],
                                    start=(kc == 0), stop=(kc == nkc - 1))
                            # evict + bias: lo half (s=2c) on ACT
                            nc.scalar.activation(
                                slab[:, 2 * c, :], ps[0:64, :],
                                AF.Identity, bias=bias[0:64, c:c + 1])
                            # hi half (s=2c+1): partition-shifted write
                            if hi_on_act:
                                nc.scalar.activation(
                                    slab[:, 2 * c + 1, :], ps[64:128, :],
                                    AF.Identity, bias=bias[64:128, c:c + 1])
                            else:
                                nc.vector.tensor_scalar_add(
                                    slab[:, 2 * c + 1, :], ps[64:128, :],
                                    bias[64:128, c:c + 1])

                # ---- attention, groups of G tokens ----
                for t0 in range(0, TB, G):
                    g = min(G, TB - t0)
                    sc = pss_pool.tile([64, G, S], f32, tag="sc")
                    for i in range(g):
                        nc.tensor.matmul(sc[:, i, :], kts[:, :, t0 + i],
                                         qts[:, :, t0 + i],
                                         start=True, stop=True)
                    eT = small_pool.tile([64, G, S], bf16, tag="eT")
                    nc.scalar.activation(eT[:, 0:g, :], sc[:, 0:g, :], AF.Exp)
                    op = pso_pool.tile([64, G, DT + 1], f32, tag="op")
                    for i in range(g):
                        nc.tensor.matmul(op[:, i, :], eT[:, i, :],
                                         vs[:, :, t0 + i],
                                         start=True, stop=True)
                    rd = small_pool.tile([64, G], f32, tag="rd")
                    nc.vector.reciprocal(rd[:, 0:g], op[:, 0:g, DT])
                    rdb = rd[:, 0:g].unsqueeze(1).broadcast_to([64, CH, g])
                    nc.vector.tensor_tensor(
                        otp[0:64, :, t0:t0 + g],
                        op[:, 0:g, 0:DT:2].transpose([0, 2, 1]), rdb, MUL)
                    nc.vector.tensor_tensor(
                        otp[64:128, :, t0:t0 + g],
                        op[:, 0:g, 1:DT:2].transpose([0, 2, 1]), rdb, MUL)

                # ---- output projection ----
                fins = [psb_pool.tile([128, TB], f32, tag="big",
                                      name=f"fin{rep}{blk}_{i}")
                        for i in range(CT)]
                for cc4 in range(CH // CB):
                    wt = wts_pool.tile([128, CB, CT, 128], bf16, tag="wo")
                    nc.sync.dma_start(
                        wt[:], wo_d[:, CB * cc4:CB * (cc4 + 1), :].rearrange(
                            "p c (ct m) -> p c ct m", m=128))
                    for ci in range(CB):
                        cc = CB * cc4 + ci
                        for ct in range(CT):
                            nc.tensor.matmul(
                                fins[ct][:], wt[:, ci, ct, :],
                                otp[:, cc, :],
                                start=(cc == 0), stop=(cc == CH - 1))
                for ct in range(CT):
                    ob = osb_pool.tile([128, TB], f32, tag="ob")
                    nc.vector.tensor_copy(ob[:], fins[ct][:])
                    nc.sync.dma_start(
                        pt_d[128 * ct:128 * (ct + 1), tsl], ob[:])

    nc.compile()
    return nc


# dt-major permutation: new index dt*S+s  <- old index s*DT+dt
_PERM = np.arange(S * DT).reshape(S, DT).T.reshape(-1)


def _prep_core_inputs(h, x, z, Wq, bq, Wk, bk, Wv, bv, Wo):
    dsl = slice(h * DH, (h + 1) * DH)

    def dev_w(w, nkc):
        # [nkc*128, DH] -> [p, c, kc*128+m]
        return np.ascontiguousarray(
            w.reshape(nkc, 128, CH, 128).transpose(1, 2, 0, 3)
            .reshape(128, CH, nkc * 128).astype(_bf16))

    wq_h = Wq[:, dsl] * np.float32(0.125)
    bq_h = bq[dsl] * np.float32(0.125)
    wk_h = Wk[:, dsl]
    bk_h = bk[dsl]
    wv_h = Wv[:, dsl][:, _PERM]
    bv_h = bv[dsl][_PERM]
    wo_h = Wo[dsl, :][_PERM, :]

    zt = z.reshape(T, Z_SIZE).T.reshape(KCQ, 128, T).transpose(1, 0, 2)
    xt = x.reshape(T, INPUT_SIZE).T.reshape(KCX, 128, T).transpose(1, 0, 2)
    return {
        "zt": np.ascontiguousarray(zt.astype(_bf16)),
        "xt": np.ascontiguousarray(xt.astype(_bf16)),
        "wq": dev_w(wq_h, KCQ),
        "wk": dev_w(wk_h, KCX),
        "wv": dev_w(wv_h, KCX),
        "wo": np.ascontiguousarray(
            wo_h.reshape(CH, 128, CT, 128).transpose(1, 0, 2, 3)
            .reshape(128, CH, CT * 128).astype(_bf16)),
        "bq": np.ascontiguousarray(bq_h.reshape(CH, 128).T.astype(np.float32)),
        "bk": np.ascontiguousarray(bk_h.reshape(CH, 128).T.astype(np.float32)),
        "bv": np.ascontiguousarray(bv_h.reshape(CH, 128).T.astype(np.float32)),
    }


def make_in_maps(x, z, Wq, bq, Wk, bk, Wv, bv, Wo):
    x = np.asarray(x, np.float32)
    z = np.asarray(z, np.float32)
    return [
        _prep_core_inputs(h, x, z, np.asarray(Wq, np.float32),
                          np.asarray(bq, np.float32), np.asarray(Wk, np.float32),
                          np.asarray(bk, np.float32), np.asarray(Wv, np.float32),
                          np.asarray(bv, np.float32), np.asarray(Wo, np.float32))
        for h in range(H)
    ]


def get_nc(reps=1):
    key = f"nc{reps}"
    if key not in _cache:
        _cache[key] = _build_nc(reps)
    return _cache[key]


def run_spmd(in_maps, trace=False):
    from concourse.bass_utils import run_bass_kernel_spmd
    nc = get_nc()
    return run_bass_kernel_spmd(nc, in_maps, list(range(H)), trace=trace)


def assemble_output(results, bo):
    total = np.zeros((INPUT_SIZE, T), np.float64)
    for r in results:
        total += r["pt"].astype(np.float64)
    out = total.T.astype(np.float32) + np.asarray(bo, np.float32)
    return np.ascontiguousarray(out.reshape(B, N, INPUT_SIZE))


def kernel(x, z, Wq, bq, Wk, bk, Wv, bv, Wo, bo):
    in_maps = make_in_maps(x, z, Wq, bq, Wk, bk, Wv, bv, Wo)
    res = run_spmd(in_maps)
    return assemble_output(res.results, bo)

